# revision 7
# baseline (speedup 1.0000x reference)
"""Multi-head attention (12 heads, d_k=64, seq 2048, batch 4) on 8 TRN2 NeuronCores.

Sharding: core c handles batch b=c//2 and query-half qh=c%2 (1024 query rows).
Each core computes K/V projections for its whole batch (2048 rows) plus Q for its
half, runs flash-style attention fully on-chip (scores never hit HBM), and writes
a disjoint [1024, 768] slice of the output -> no cross-core reduction needed.

Trick: the query half is selected host-side by rotating x so the core's query
rows are always rows 0..1024 (attention is permutation-invariant over keys), so
all 8 cores run one SPMD program.

Layouts (SBUF):
  xT   [768(d) x 2048(s)]   as [128, 6*2048] f32   (PE-transposed on chip)
  W*T  [768(d) x 768(e)]    as [128, 6*768]  f32   (PE-transposed on chip)
  qT   [768(e) x 1024(s)]   as [128, 6*1024] bf16  (head h: chunk h//2, parts (h%2)*64..)
  kT   [768(e) x 2048(s)]   as [128, 6*2048] bf16
  v    [2048(s) x 12*65]    as [128, 16*780] bf16  (per head: 64 v cols + ones col
                                                    -> ctx matmul also accumulates the
                                                    softmax denominator as row 64)
Attention per head (S^T layout, keys on partitions):
  S^T[j,i] = k_h^T(lhsT) . q_h^T(rhs), K=64        -> PSUM [128,1024]
  P^T = exp(S^T/8)  (no max subtraction; |scores|<~8 so exp is safe in fp32)
  ctx^T[d,i] (+denom row) = v_aug(lhsT) . P^T(rhs), K=128-chunks, accum over j
  normalize: r = 1/denom, broadcast across partitions via DMA, multiply.
Out projection: out[s,e] = ctxT(lhsT) . WoT(rhs), + bias.

fp32 tensors are fed to the PE as float32r (full-rate for N>=256).
"""

import sys

import numpy as np

if "/opt/trn_rl_repo" not in sys.path:
    sys.path.insert(0, "/opt/trn_rl_repo")

import concourse.bass as bass  # noqa: F401  (registers engine methods)
import concourse.tile as tile
from concourse import bacc, bass_utils, mybir
from concourse.masks import make_identity

HIDDEN, HEADS, DK = 768, 12, 64
BS, SEQ = 4, 2048
NCORES = 8
Q = SEQ // 2          # query rows per core
DC = HIDDEN // 128    # 6 chunks over d / e
SC = SEQ // 128       # 16 chunks over s (keys)
QC = Q // 128         # 8 chunks over query rows
HV = DK + 1           # per-head v block width incl. ones column

F32 = mybir.dt.float32
F32R = mybir.dt.float32r
BF16 = mybir.dt.bfloat16
EXP = mybir.ActivationFunctionType.Exp

INCLUDE_BIAS = True
ATT_DT = BF16          # storage dtype for qT/kT/v/pT
PROJ_DT = F32R         # storage dtype for matmul-feeding fp32 tensors

W_NAMES = ("Wq", "Wk", "Wv", "Wo")
B_NAMES = ("bq", "bk", "bv", "bo")


def _r(ap):
    """fp32 AP -> float32r view (full-rate PE matmul for N>=256)."""
    return ap.bitcast(F32R)


def _n_splits(total):
    """Split a free dim into <=512 matmul slices."""
    out, off = [], 0
    while off < total:
        nn = min(512, total - off)
        out.append((off, nn))
        off += nn
    return out


def _load_wT(tc, nc, pool, w_ap, ident, name):
    """Load W [e,d] from DRAM and PE-transpose to W.T chunks [128(d), DC*768(e)].

    Chunk k (d rows k*128..) lives at cols [k*768, (k+1)*768).
    """
    wT = pool.tile([128, DC * HIDDEN], PROJ_DT, tag=name, name=name)
    with tc.tile_pool(name=f"{name}_stg", bufs=2) as stg, \
         tc.tile_pool(name=f"{name}_tps", bufs=4, space="PSUM") as tps:
        for rr in range(DC):  # e-chunks (rows of W)
            wn = stg.tile([128, HIDDEN], F32, tag="wn", name=f"{name}_nat{rr}")
            nc.sync.dma_start(wn, w_ap[rr * 128:(rr + 1) * 128, :])
            for cc in range(DC):  # d-chunks (cols of W)
                ps = tps.tile([128, 128], F32, tag="tp", name=f"{name}_tp{rr}_{cc}")
                nc.tensor.transpose(ps, wn[:, cc * 128:(cc + 1) * 128], ident)
                nc.vector.tensor_copy(
                    wT[:, cc * HIDDEN + rr * 128: cc * HIDDEN + (rr + 1) * 128], ps)
    return wT


def _emit(tc, aps):
    nc = tc.nc
    x_ap, out_ap = aps["x"], aps["out"]

    with tc.tile_pool(name="const", bufs=1) as const, \
         tc.tile_pool(name="pers", bufs=1) as pers:
        ident = const.tile([128, 128], F32, tag="ident", name="ident")
        make_identity(nc, ident)
        ones_f32 = const.tile([1, 512], F32, tag="ones32", name="ones_f32")
        nc.vector.memset(ones_f32, 1.0)
        ones_row = const.tile([1, 512], PROJ_DT, tag="ones", name="ones_row")
        nc.vector.tensor_copy(ones_row, ones_f32)
        if INCLUDE_BIAS:
            bias_f32 = const.tile([1, 4 * HIDDEN], F32, tag="bias32", name="bias_f32")
            for i, bn in enumerate(B_NAMES):
                nc.sync.dma_start(
                    bias_f32[0:1, i * HIDDEN:(i + 1) * HIDDEN], aps[bn][None, :])
            bias_sb = const.tile([1, 4 * HIDDEN], PROJ_DT, tag="bias", name="bias_sb")
            nc.vector.tensor_copy(bias_sb, bias_f32)

        kT = pers.tile([128, DC * SEQ], ATT_DT, tag="kT", name="kT")
        qT = pers.tile([128, DC * Q], ATT_DT, tag="qT", name="qT")
        vv = pers.tile([128, SC * HEADS * HV], ATT_DT, tag="vv", name="vv")
        ctxT = pers.tile([128, DC * Q], PROJ_DT, tag="ctxT", name="ctxT")
        vv_heads = vv.rearrange("p (g c) -> p g c", c=HV)

        with tc.tile_pool(name="xT_pool", bufs=1) as xtp:
            xT = xtp.tile([128, DC * SEQ], PROJ_DT, tag="xT", name="xT")

            # ---- Phase A: load x, transpose to xT ------------------------------
            with tc.tile_pool(name="xstg", bufs=3) as xstg, \
                 tc.tile_pool(name="xtps", bufs=4, space="PSUM") as xtps:
                for st in range(SC):
                    xn = xstg.tile([128, HIDDEN], F32, tag="xn", name=f"xnat{st}")
                    nc.sync.dma_start(xn, x_ap[st * 128:(st + 1) * 128, :])
                    for cc in range(DC):
                        ps = xtps.tile([128, 128], F32, tag="tp", name=f"xtp{st}_{cc}")
                        nc.tensor.transpose(ps, xn[:, cc * 128:(cc + 1) * 128], ident)
                        nc.vector.tensor_copy(
                            xT[:, cc * SEQ + st * 128: cc * SEQ + (st + 1) * 128], ps)

            # ---- Phase B1/C1: Wq,Wk -> qT, kT ---------------------------------
            with tc.tile_pool(name="wqk", bufs=1) as wqk, \
                 tc.tile_pool(name="qk_ps", bufs=2, space="PSUM") as qkps:
                wqT = _load_wT(tc, nc, wqk, aps["Wq"], ident, "wqT")
                wkT = _load_wT(tc, nc, wqk, aps["Wk"], ident, "wkT")

                # kT: out[e-chunk m, s] ; lhsT = WkT[d, e-chunk], rhs = xT[d, s]
                for m in range(DC):
                    for sh in range(SEQ // 1024):
                        ps = qkps.tile([128, 1024], F32, tag="pps", name=f"kps{m}_{sh}")
                        for n2 in range(2):
                            ncol = sh * 1024 + n2 * 512
                            for k in range(DC):
                                nc.tensor.matmul(
                                    ps[:, n2 * 512:(n2 + 1) * 512],
                                    (wkT[:, k * HIDDEN + m * 128: k * HIDDEN + (m + 1) * 128]),
                                    (xT[:, k * SEQ + ncol: k * SEQ + ncol + 512]),
                                    start=(k == 0),
                                    stop=(k == DC - 1 and not INCLUDE_BIAS))
                            if INCLUDE_BIAS:
                                nc.tensor.matmul(
                                    ps[:, n2 * 512:(n2 + 1) * 512],
                                    (bias_sb[0:1, HIDDEN + m * 128: HIDDEN + (m + 1) * 128]),
                                    (ones_row[0:1, 0:512]),
                                    start=False, stop=True)
                        nc.vector.tensor_copy(
                            kT[:, m * SEQ + sh * 1024: m * SEQ + (sh + 1) * 1024], ps)

                # qT: query rows are rows 0..Q of (rotated) x
                for m in range(DC):
                    ps = qkps.tile([128, 1024], F32, tag="pps", name=f"qps{m}")
                    for n2 in range(2):
                        ncol = n2 * 512
                        for k in range(DC):
                            nc.tensor.matmul(
                                ps[:, n2 * 512:(n2 + 1) * 512],
                                (wqT[:, k * HIDDEN + m * 128: k * HIDDEN + (m + 1) * 128]),
                                (xT[:, k * SEQ + ncol: k * SEQ + ncol + 512]),
                                start=(k == 0),
                                stop=(k == DC - 1 and not INCLUDE_BIAS))
                        if INCLUDE_BIAS:
                            nc.tensor.matmul(
                                ps[:, n2 * 512:(n2 + 1) * 512],
                                (bias_sb[0:1, m * 128:(m + 1) * 128]),
                                (ones_row[0:1, 0:512]),
                                start=False, stop=True)
                    nc.vector.tensor_copy(qT[:, m * Q:(m + 1) * Q], ps)

            # ---- Phase B2/C2: Wv -> v (natural layout, head-strided + ones) ----
            with tc.tile_pool(name="wv", bufs=1) as wvp, \
                 tc.tile_pool(name="v_ps", bufs=2, space="PSUM") as vps:
                wvT = _load_wT(tc, nc, wvp, aps["Wv"], ident, "wvT")
                # ones columns of v-aug (denominator accumulators), written once
                nc.vector.memset(vv_heads[:, :, DK:DK + 1], 1.0)
                for st in range(SC):
                    ps = vps.tile([128, HIDDEN], F32, tag="vps", name=f"vps{st}")
                    for (n0, nn) in _n_splits(HIDDEN):
                        for k in range(DC):
                            nc.tensor.matmul(
                                ps[:, n0:n0 + nn],
                                (xT[:, k * SEQ + st * 128: k * SEQ + (st + 1) * 128]),
                                (wvT[:, k * HIDDEN + n0: k * HIDDEN + n0 + nn]),
                                start=(k == 0),
                                stop=(k == DC - 1 and not INCLUDE_BIAS))
                        if INCLUDE_BIAS:
                            nc.tensor.matmul(
                                ps[:, n0:n0 + nn],
                                (ones_row[0:1, 0:128]),
                                (bias_sb[0:1, 2 * HIDDEN + n0: 2 * HIDDEN + n0 + nn]),
                                start=False, stop=True)
                    nc.vector.tensor_copy(
                        vv_heads[:, st * HEADS:(st + 1) * HEADS, 0:DK],
                        ps.rearrange("p (g c) -> p g c", c=DK))

        # xT freed here.

        # ---- Phase D: attention + out projection ------------------------------
        with tc.tile_pool(name="wo", bufs=1) as wop:
            woT = _load_wT(tc, nc, wop, aps["Wo"], ident, "woT")

            with tc.tile_pool(name="st_ps", bufs=2, space="PSUM") as stp, \
                 tc.tile_pool(name="ctx_ps", bufs=2, space="PSUM") as cxp, \
                 tc.tile_pool(name="ptp", bufs=1) as ptp, \
                 tc.tile_pool(name="rrp", bufs=2) as rrp:
                pt = ptp.tile([128, SC * Q], ATT_DT, tag="pt", name="pt")
                for h in range(HEADS):
                    ch, off = h // 2, (h % 2) * DK
                    # S^T then exp, all 16 key-chunks (keeps PE in 64-row mode)
                    for jc in range(SC):
                        stt = stp.tile([128, Q], F32, tag="st", name=f"st{h}_{jc}")
                        for n2 in range(2):
                            nc.tensor.matmul(
                                stt[:, n2 * 512:(n2 + 1) * 512],
                                kT[off:off + DK, ch * SEQ + jc * 128: ch * SEQ + (jc + 1) * 128],
                                qT[off:off + DK, ch * Q + n2 * 512: ch * Q + n2 * 512 + 512],
                                start=True, stop=True)
                        nc.scalar.activation(
                            pt[:, jc * Q:(jc + 1) * Q], stt, EXP, scale=0.125)
                    # ctx^T (+ denominator in row 64), accumulated over key chunks
                    ctx_ps = cxp.tile([HV, Q], F32, tag="ctx", name=f"ctx{h}")
                    for jc in range(SC):
                        for n2 in range(2):
                            nc.tensor.matmul(
                                ctx_ps[:, n2 * 512:(n2 + 1) * 512],
                                vv[:, (jc * HEADS + h) * HV: (jc * HEADS + h + 1) * HV],
                                pt[:, jc * Q + n2 * 512: jc * Q + n2 * 512 + 512],
                                start=(jc == 0), stop=(jc == SC - 1))
                    # normalize: r = 1/denom, broadcast down partitions, multiply
                    r_row = rrp.tile([1, Q], F32, tag="rrow", name=f"rrow{h}")
                    nc.vector.reciprocal(r_row, ctx_ps[DK:DK + 1, :])
                    rb = rrp.tile([DK, Q], F32, tag="rb", name=f"rb{h}")
                    nc.gpsimd.partition_broadcast(rb, r_row)
                    nc.vector.tensor_mul(
                        ctxT[off:off + DK, ch * Q:(ch + 1) * Q],
                        ctx_ps[0:DK, :], rb)

            # ---- Phase E: out projection --------------------------------------
            with tc.tile_pool(name="o_ps", bufs=2, space="PSUM") as ops_, \
                 tc.tile_pool(name="o_sb", bufs=3) as osb:
                for m in range(QC):
                    po = ops_.tile([128, HIDDEN], F32, tag="po", name=f"po{m}")
                    for (n0, nn) in _n_splits(HIDDEN):
                        for k in range(DC):
                            nc.tensor.matmul(
                                po[:, n0:n0 + nn],
                                (ctxT[:, k * Q + m * 128: k * Q + (m + 1) * 128]),
                                (woT[:, k * HIDDEN + n0: k * HIDDEN + n0 + nn]),
                                start=(k == 0),
                                stop=(k == DC - 1 and not INCLUDE_BIAS))
                        if INCLUDE_BIAS:
                            nc.tensor.matmul(
                                po[:, n0:n0 + nn],
                                (ones_row[0:1, 0:128]),
                                (bias_sb[0:1, 3 * HIDDEN + n0: 3 * HIDDEN + n0 + nn]),
                                start=False, stop=True)
                    ot = osb.tile([128, HIDDEN], F32, tag="ot", name=f"ot{m}")
                    nc.vector.tensor_copy(ot, po)
                    nc.sync.dma_start(out_ap[m * 128:(m + 1) * 128, :], ot)


def build():
    nc = bacc.Bacc("TRN2", target_bir_lowering=False, debug=False,
                   num_devices=NCORES)
    aps = {"x": nc.dram_tensor("x", [SEQ, HIDDEN], F32, kind="ExternalInput").ap()}
    for nm in W_NAMES:
        aps[nm] = nc.dram_tensor(nm, [HIDDEN, HIDDEN], F32, kind="ExternalInput").ap()
    for nm in B_NAMES:
        aps[nm] = nc.dram_tensor(nm, [HIDDEN], F32, kind="ExternalInput").ap()
    aps["out"] = nc.dram_tensor("out", [Q, HIDDEN], F32, kind="ExternalOutput").ap()
    with tile.TileContext(nc) as tc:
        _emit(tc, aps)
    nc.compile()
    return nc


_NC_CACHE = None


def _get_nc():
    global _NC_CACHE
    if _NC_CACHE is None:
        _NC_CACHE = build()
    return _NC_CACHE


def make_in_maps(x, Wq, bq, Wk, bk, Wv, bv, Wo, bo):
    f = lambda a: np.ascontiguousarray(np.asarray(a, dtype=np.float32))
    x = f(x)
    shared = dict(Wq=f(Wq), bq=f(bq), Wk=f(Wk), bk=f(bk),
                  Wv=f(Wv), bv=f(bv), Wo=f(Wo), bo=f(bo))
    in_maps = []
    for c in range(NCORES):
        b, qh = divmod(c, 2)
        xb = x[b] if qh == 0 else np.concatenate([x[b, Q:], x[b, :Q]], axis=0)
        in_maps.append(dict(x=np.ascontiguousarray(xb), **shared))
    return in_maps


def assemble(results):
    out = np.empty((BS, SEQ, HIDDEN), np.float32)
    for c in range(NCORES):
        b, qh = divmod(c, 2)
        out[b, qh * Q:(qh + 1) * Q] = results[c]["out"]
    return out


def run(in_maps, **kwargs):
    nc = _get_nc()
    return bass_utils.run_bass_kernel_spmd(
        nc, in_maps, core_ids=list(range(NCORES)), **kwargs)


def kernel(x, Wq, bq, Wk, bk, Wv, bv, Wo, bo):
    in_maps = make_in_maps(x, Wq, bq, Wk, bk, Wv, bv, Wo, bo)
    res = run(in_maps)
    return assemble(res.results)
